# revision 25
# baseline (speedup 1.0000x reference)
"""MultiHeadLatentAttention (MLA) Trainium2 Bass kernel, 8-core SPMD.

Sharding: cores 0-3 -> batch 0, cores 4-7 -> batch 1. Within a batch group,
core g owns query token rows [g*512, (g+1)*512). Phase 1 computes all
projections for the core's own 512 tokens in transposed (channel-major)
layout; an intra-group AllGather shares K/V; attention + W_o produce the
core's own output rows directly (no final reduction).

All matmuls run in float32r (full PE rate, ~1.5e-4 rel err per matmul).
"""
import numpy as np
from contextlib import ExitStack

import concourse.tile as tile
import concourse.mybir as mybir
from concourse import bacc
from concourse.bass_utils import run_bass_kernel_spmd
from concourse.masks import make_identity

P = 128
B, T = 2, 2048
D_MODEL, N_HEADS, D_HEAD = 2048, 16, 128
D_LATENT, D_LATENT_Q = 512, 1536
D_ROPE = 64
ROPE_BASE = 10000.0
N_CORES = 8
G = 4                      # cores per batch group
TQ = T // G                # 512 own query tokens per core
SCALE = 1.0 / float(np.sqrt(D_HEAD + D_ROPE))

F32 = mybir.dt.float32
F32R = mybir.dt.float32r

# flat collective buffer layout (elements, per rank)
KC_SZ = D_MODEL * TQ           # k_cT  [2048, 512] channel-major
KR_SZ = D_ROPE * TQ            # k_rT  [64, 512]
V_SZ = TQ * D_MODEL            # v     [512, 2048] token-major
CC_SZ = KC_SZ + KR_SZ + V_SZ


def _build(apply_mask: bool):
    nc = bacc.Bacc("TRN2", target_bir_lowering=False, debug=False,
                   num_devices=N_CORES)

    h_own = nc.dram_tensor("h_own", [TQ, D_MODEL], F32, kind="ExternalInput").ap()
    w_dq = nc.dram_tensor("w_dq", [D_MODEL, D_LATENT_Q], F32R, kind="ExternalInput").ap()
    w_uq = nc.dram_tensor("w_uq", [D_LATENT_Q, D_MODEL], F32R, kind="ExternalInput").ap()
    w_dkv = nc.dram_tensor("w_dkv", [D_MODEL, D_LATENT], F32R, kind="ExternalInput").ap()
    w_uk = nc.dram_tensor("w_uk", [D_LATENT, D_MODEL], F32R, kind="ExternalInput").ap()
    w_uv = nc.dram_tensor("w_uv", [D_LATENT, D_MODEL], F32R, kind="ExternalInput").ap()
    w_qr = nc.dram_tensor("w_qr", [D_LATENT_Q, N_HEADS * D_ROPE], F32R, kind="ExternalInput").ap()
    w_kr = nc.dram_tensor("w_kr", [D_MODEL, D_ROPE], F32R, kind="ExternalInput").ap()
    w_o = nc.dram_tensor("w_o", [D_MODEL, D_MODEL], F32R, kind="ExternalInput").ap()
    # cos/sin replicated 4x along partitions: row p holds table row p % 32
    cos_t = nc.dram_tensor("cos_t", [P, TQ], F32, kind="ExternalInput").ap()
    sin_t = nc.dram_tensor("sin_t", [P, TQ], F32, kind="ExternalInput").ap()
    if apply_mask:
        # mask[0,0][own_q_rows, :].T / SCALE  -> [k, q] layout
        mask_t = nc.dram_tensor("mask_t", [T, TQ], F32, kind="ExternalInput").ap()
    out = nc.dram_tensor("out", [TQ, D_MODEL], F32, kind="ExternalOutput").ap()

    cc_in = nc.dram_tensor("cc_in", [CC_SZ], F32R)
    cc_out = nc.dram_tensor("cc_out", [G * CC_SZ], F32R)

    def kc_g(r):
        return cc_out[r * CC_SZ: r * CC_SZ + KC_SZ].rearrange("(c t) -> c t", t=TQ)

    def kr_g(r):
        o = r * CC_SZ + KC_SZ
        return cc_out[o: o + KR_SZ].rearrange("(c t) -> c t", t=TQ)

    def v_g(r):
        o = r * CC_SZ + KC_SZ + KR_SZ
        return cc_out[o: o + V_SZ].rearrange("(t c) -> t c", c=D_MODEL)

    with ExitStack() as ctx:
        tc = ctx.enter_context(tile.TileContext(nc))
        const = ctx.enter_context(tc.tile_pool(name="const", bufs=1))
        big = ctx.enter_context(tc.tile_pool(name="big", bufs=1))
        wpool = ctx.enter_context(tc.tile_pool(name="wpool", bufs=2))
        work = ctx.enter_context(tc.tile_pool(name="work", bufs=3))
        ps_proj = ctx.enter_context(tc.tile_pool(name="ps_proj", bufs=3, space="PSUM"))
        ps_sc = ctx.enter_context(tc.tile_pool(name="ps_sc", bufs=2, space="PSUM"))
        ps_den = ctx.enter_context(tc.tile_pool(name="ps_den", bufs=1, space="PSUM"))
        ps_pv = ctx.enter_context(tc.tile_pool(name="ps_pv", bufs=2, space="PSUM"))

        ident = const.tile([P, P], F32)
        make_identity(nc, ident[:])
        ones_f32 = const.tile([P, 1], F32)
        nc.vector.memset(ones_f32[:], 1.0)
        ones_col = const.tile([P, 1], F32R)
        nc.vector.tensor_copy(ones_col[:], ones_f32[:])
        ones_rf = const.tile([1, P], F32)
        nc.vector.memset(ones_rf[:], 1.0)
        ones_row = const.tile([1, P], F32R)
        nc.vector.tensor_copy(ones_row[:], ones_rf[:])
        cosb = const.tile([P, TQ], F32)
        nc.sync.dma_start(cosb[:], cos_t)
        sinb = const.tile([P, TQ], F32)
        nc.sync.dma_start(sinb[:], sin_t)

        # ---- load h in chunks and transpose to channel-major hT [2048, 512] ----
        hT = big.tile([P, D_MODEL // P, TQ], F32R, tag="hT")
        for i in range(TQ // P):
            h_sb = wpool.tile([P, D_MODEL], F32, tag="wA", name=f"h_sb_{i}")
            nc.sync.dma_start(h_sb[:], h_own[i * P:(i + 1) * P, :])
            for j in range(D_MODEL // P):
                pst = ps_proj.tile([P, P], F32, tag="proj", name=f"pst_{i}_{j}")
                nc.tensor.transpose(pst[:], h_sb[:, j * P:(j + 1) * P], ident[:])
                nc.vector.tensor_copy(hT[:, j, i * P:(i + 1) * P], pst[:])

        # generic projection: out_T[MD, TQ] = W[KD, MD].T @ actT   (lhsT = W)
        # actT: sbuf tile [128, KD//128, TQ] f32r. out_tile: [128, MD//128, TQ].
        def project(w_ap, KD, MD, actT, out_tile, name):
            for m in range(MD // P):
                wt = wpool.tile([P, KD // P, P], F32R, tag="wA", name=f"w_{name}_{m}")
                nc.sync.dma_start(
                    wt[:], w_ap[:, m * P:(m + 1) * P].rearrange("(ko p) m -> p ko m", p=P))
                ps = ps_proj.tile([P, TQ], F32, tag="proj", name=f"ps_{name}_{m}")
                for k in range(KD // P):
                    nc.tensor.matmul(ps[:], wt[:, k], actT[:, k],
                                     start=(k == 0), stop=(k == KD // P - 1))
                nc.vector.tensor_copy(out_tile[:, m], ps[:])

        # ---- KV chain (own tokens) ----
        c_kvT = big.tile([P, D_LATENT // P, TQ], F32R, tag="lat")
        project(w_dkv, D_MODEL, D_LATENT, hT, c_kvT, "dkv")

        k_cT = big.tile([P, D_MODEL // P, TQ], F32R, tag="kcT")
        project(w_uk, D_LATENT, D_MODEL, c_kvT, k_cT, "uk")
        nc.sync.dma_start(
            cc_in[0:KC_SZ].rearrange("(o p t) -> p o t", p=P, t=TQ), k_cT[:])

        # k_r: raw [64, 512] then rope (w_kr host-permuted: even js then odd js).
        # DVE ops need equal base partitions, so stage x2 down to base 0 via
        # SBUF->SBUF DMA (DMA can shift partitions; DVE cannot).
        krt_raw = work.tile([D_ROPE, TQ], F32, tag="rt_a", bufs=2, name="krt_raw")
        wt_kr = wpool.tile([P, D_MODEL // P, D_ROPE], F32R, tag="wA", name="wkr")
        nc.sync.dma_start(wt_kr[:], w_kr.rearrange("(ko p) m -> p ko m", p=P))
        ps_kr = ps_proj.tile([D_ROPE, TQ], F32, tag="proj", name="ps_kr")
        for k in range(D_MODEL // P):
            nc.tensor.matmul(ps_kr[:], wt_kr[:, k], hT[:, k],
                             start=(k == 0), stop=(k == D_MODEL // P - 1))
        nc.vector.tensor_copy(krt_raw[:], ps_kr[:])
        k_rT = work.tile([D_ROPE, TQ], F32R, tag="krT", bufs=1)
        kx2 = work.tile([32, TQ], F32, tag="kx2", bufs=2)
        nc.sync.dma_start(kx2[:], krt_raw[32:64])
        kt1 = work.tile([32, TQ], F32, tag="rt_a", bufs=2)
        kt2 = work.tile([32, TQ], F32, tag="rt_b", bufs=2)
        ko2 = work.tile([32, TQ], F32R, tag="kx2", bufs=2)
        nc.vector.tensor_mul(kt1[:], krt_raw[0:32], cosb[0:32])
        nc.vector.tensor_mul(kt2[:], kx2[:], sinb[0:32])
        nc.vector.tensor_sub(k_rT[0:32], kt1[:], kt2[:])
        nc.vector.tensor_mul(kt1[:], krt_raw[0:32], sinb[0:32])
        nc.vector.tensor_mul(kt2[:], kx2[:], cosb[0:32])
        nc.vector.tensor_add(ko2[:], kt1[:], kt2[:])
        nc.sync.dma_start(k_rT[32:64], ko2[:])
        nc.sync.dma_start(
            cc_in[KC_SZ:KC_SZ + KR_SZ].rearrange("(p t) -> p t", t=TQ), k_rT[:])

        # v token-major [512, 2048]: lhsT = c_kvT m-slices, rhs = w_uv n-slices
        v_view = cc_in[KC_SZ + KR_SZ:CC_SZ].rearrange("(t c) -> t c", c=D_MODEL)
        for n in range(D_MODEL // 512):
            wv = wpool.tile([P, D_LATENT // P, 512], F32R, tag="wA", name=f"wuv_{n}")
            nc.sync.dma_start(
                wv[:], w_uv[:, n * 512:(n + 1) * 512].rearrange("(ko p) m -> p ko m", p=P))
            for m in range(TQ // P):
                ps = ps_proj.tile([P, 512], F32, tag="proj", name=f"ps_v_{n}_{m}")
                for k in range(D_LATENT // P):
                    nc.tensor.matmul(ps[:], c_kvT[:, k, m * P:(m + 1) * P], wv[:, k],
                                     start=(k == 0), stop=(k == D_LATENT // P - 1))
                vtile = work.tile([P, 512], F32R, tag="vtile", name=f"vt_{n}_{m}", bufs=2)
                nc.vector.tensor_copy(vtile[:], ps[:])
                nc.sync.dma_start(
                    v_view[m * P:(m + 1) * P, n * 512:(n + 1) * 512], vtile[:])

        # ---- AllGather K/V within batch group ----
        nc.gpsimd.collective_compute(
            "AllGather", mybir.AluOpType.bypass,
            ins=[cc_in[:]], outs=[cc_out[:]],
            replica_groups=[[0, 1, 2, 3], [4, 5, 6, 7]],
        )

        # ---- Q chain (own tokens), overlaps the gather ----
        c_qT = big.tile([P, D_LATENT_Q // P, TQ], F32R, tag="cqT")
        project(w_dq, D_MODEL, D_LATENT_Q, hT, c_qT, "dq")

        q_cT = big.tile([P, D_MODEL // P, TQ], F32R, tag="kcT")
        project(w_uq, D_LATENT_Q, D_MODEL, c_qT, q_cT, "uq")

        # W_qr is host-permuted so output channels 0-511 are x1 (even rope dims,
        # head-major) and 512-1023 are x2. Rope then runs on full 128-partition
        # tiles: o1[kt] = x1[kt]*cos - x2[kt]*sin, o2[kt] = x1[kt]*sin + x2[kt]*cos.
        q_rT_raw = big.tile([P, (N_HEADS * D_ROPE) // P, TQ], F32, tag="lat")
        project(w_qr, D_LATENT_Q, N_HEADS * D_ROPE, c_qT, q_rT_raw, "qr")
        qro1 = big.tile([P, 4, TQ], F32R, tag="qro1")
        qro2 = big.tile([P, 4, TQ], F32R, tag="qro2")
        for kt in range(4):
            x1 = q_rT_raw[:, kt]
            x2 = q_rT_raw[:, 4 + kt]
            t1 = work.tile([P, TQ], F32, tag="rt_a", name=f"rt_a_{kt}", bufs=2)
            t2 = work.tile([P, TQ], F32, tag="rt_b", name=f"rt_b_{kt}", bufs=2)
            nc.vector.tensor_mul(t1[:], x1, cosb[:])
            nc.vector.tensor_mul(t2[:], x2, sinb[:])
            nc.vector.tensor_sub(qro1[:, kt], t1[:], t2[:])
            nc.vector.tensor_mul(t1[:], x1, sinb[:])
            nc.vector.tensor_mul(t2[:], x2, cosb[:])
            nc.vector.tensor_add(qro2[:, kt], t1[:], t2[:])

        # ---- attention (scoresT layout [k, q]; no transposes needed) ----
        if apply_mask:
            mask_sb = big.tile([P, T // P, TQ], F32, tag="hT")
            nc.sync.dma_start(mask_sb[:], mask_t.rearrange("(o p) q -> p o q", p=P))

        attn_oT = big.tile([P, N_HEADS, TQ], F32R, tag="cqT")

        NKB = T // P  # 16 key blocks
        for h in range(N_HEADS):
            pv_ps = ps_pv.tile([P, TQ], F32, tag="pv", name=f"pv_{h}")
            den_ps = ps_den.tile([1, TQ], F32, tag="den", name=f"den_{h}")
            tl, po = h // 4, (h * 32) % P
            qr_stage = work.tile([D_ROPE, TQ], F32R, tag="qrs", name=f"qrs_{h}", bufs=2)
            nc.sync.dma_start(qr_stage[0:32], qro1[po:po + 32, tl])
            nc.sync.dma_start(qr_stage[32:64], qro2[po:po + 32, tl])
            for kb in range(NKB):
                r, loc = kb // (TQ // P), (kb % (TQ // P)) * P
                kct = work.tile([P, P], F32R, tag="kct", name=f"kct_{h}_{kb}")
                nc.sync.dma_start(kct[:], kc_g(r)[h * P:(h + 1) * P, loc:loc + P])
                krt = work.tile([D_ROPE, P], F32R, tag="krt", name=f"krt_{h}_{kb}")
                nc.sync.dma_start(krt[:], kr_g(r)[:, loc:loc + P])
                vt = work.tile([P, P], F32R, tag="vt", name=f"vt_{h}_{kb}")
                nc.sync.dma_start(vt[:], v_g(r)[loc:loc + P, h * P:(h + 1) * P])

                sc_ps = ps_sc.tile([P, TQ], F32, tag="sc", name=f"sc_{h}_{kb}")
                nc.tensor.matmul(sc_ps[:], kct[:], q_cT[:, h], start=True, stop=False)
                nc.tensor.matmul(sc_ps[:], krt[:], qr_stage[:],
                                 start=False, stop=True)
                if apply_mask:
                    nc.vector.tensor_add(sc_ps[:], sc_ps[:], mask_sb[:, kb])
                pT = work.tile([P, TQ], F32R, tag="pT", name=f"pT_{h}_{kb}")
                nc.scalar.activation(pT[:], sc_ps[:],
                                     mybir.ActivationFunctionType.Exp, scale=SCALE)
                nc.tensor.matmul(den_ps[:], ones_col[:], pT[:],
                                 start=(kb == 0), stop=(kb == NKB - 1))
                nc.tensor.matmul(pv_ps[:], vt[:], pT[:],
                                 start=(kb == 0), stop=(kb == NKB - 1))
            recip = work.tile([1, TQ], F32R, tag="recip", name=f"recip_{h}", bufs=2)
            with nc.allow_low_precision(reason="f32r recip for PE broadcast"):
                nc.vector.reciprocal(recip[:], den_ps[:])
            # broadcast recip across partitions via K=1 outer product on PE
            den_bc = ps_sc.tile([P, TQ], F32, tag="sc", name=f"dbc_{h}")
            nc.tensor.matmul(den_bc[:], ones_row[:], recip[:], start=True, stop=True)
            den_sb = work.tile([P, TQ], F32, tag="rt_a", name=f"densb_{h}", bufs=2)
            nc.vector.tensor_copy(den_sb[:], den_bc[:])
            nc.vector.tensor_mul(attn_oT[:, h], pv_ps[:], den_sb[:])

        # ---- W_o: out[tok, 2048] = attn_oT.T @ W_o (256-wide column slices) ----
        out_v = out.rearrange("(i p) c -> p i c", p=P)
        NW = 256
        for n in range(D_MODEL // NW):
            wo = wpool.tile([P, D_MODEL // P, NW], F32R, tag="wA", name=f"wo_{n}")
            nc.sync.dma_start(
                wo[:], w_o[:, n * NW:(n + 1) * NW].rearrange("(ko p) m -> p ko m", p=P))
            for m in range(TQ // P):
                ps = ps_proj.tile([P, NW], F32, tag="proj", name=f"ps_o_{n}_{m}")
                for k in range(D_MODEL // P):
                    nc.tensor.matmul(ps[:], attn_oT[:, k, m * P:(m + 1) * P],
                                     wo[:, k], start=(k == 0), stop=(k == D_MODEL // P - 1))
                otile = work.tile([P, NW], F32, tag="rt_b", name=f"ot_{n}_{m}", bufs=2)
                nc.vector.tensor_copy(otile[:], ps[:])
                nc.sync.dma_start(out_v[:, m, n * NW:(n + 1) * NW], otile[:])

    nc.compile()
    return nc


def _rope_perm_q():
    # global: all heads' even rope dims (head-major), then all odd rope dims
    evens = [h * D_ROPE + 2 * j for h in range(N_HEADS) for j in range(D_ROPE // 2)]
    odds = [h * D_ROPE + 2 * j + 1 for h in range(N_HEADS) for j in range(D_ROPE // 2)]
    return np.array(evens + odds)


def _rope_perm_k():
    # single head: evens then odds
    return np.array([2 * j for j in range(D_ROPE // 2)]
                    + [2 * j + 1 for j in range(D_ROPE // 2)])


def kernel(**inputs) -> np.ndarray:
    hidden = np.ascontiguousarray(np.asarray(inputs["hidden_states"], dtype=np.float32))
    mask = np.asarray(inputs["attention_mask"], dtype=np.float32)[0, 0]   # [T, T]
    apply_mask = bool(np.any(mask != 0.0))

    w_qr = np.ascontiguousarray(np.asarray(inputs["W_qr"], np.float32)[:, _rope_perm_q()])
    w_kr = np.ascontiguousarray(np.asarray(inputs["W_kr"], np.float32)[:, _rope_perm_k()])

    freqs = 1.0 / (ROPE_BASE ** (np.arange(0, D_ROPE, 2, dtype=np.float32) / D_ROPE))
    angles = np.outer(np.arange(T, dtype=np.float32), freqs)   # [T, 32]
    cos_full = np.tile(np.cos(angles).T.astype(np.float32), (4, 1))   # [128, T]
    sin_full = np.tile(np.sin(angles).T.astype(np.float32), (4, 1))

    shared = {
        "w_dq": np.ascontiguousarray(inputs["W_dq"], np.float32),
        "w_uq": np.ascontiguousarray(inputs["W_uq"], np.float32),
        "w_dkv": np.ascontiguousarray(inputs["W_dkv"], np.float32),
        "w_uk": np.ascontiguousarray(inputs["W_uk"], np.float32),
        "w_uv": np.ascontiguousarray(inputs["W_uv"], np.float32),
        "w_qr": w_qr,
        "w_kr": w_kr,
        "w_o": np.ascontiguousarray(inputs["W_o"], np.float32),
    }

    nc = _build(apply_mask)
    in_maps = []
    for core in range(N_CORES):
        b, g = core // G, core % G
        rows = slice(g * TQ, (g + 1) * TQ)
        m = dict(shared)
        m["h_own"] = np.ascontiguousarray(hidden[b, rows])
        m["cos_t"] = np.ascontiguousarray(cos_full[:, rows])
        m["sin_t"] = np.ascontiguousarray(sin_full[:, rows])
        if apply_mask:
            m["mask_t"] = np.ascontiguousarray(mask[rows].T / SCALE)
        in_maps.append(m)

    res = run_bass_kernel_spmd(nc, in_maps, core_ids=list(range(N_CORES)))

    out = np.empty((B, T, D_MODEL), dtype=np.float32)
    for core in range(N_CORES):
        b, g = core // G, core % G
        out[b, g * TQ:(g + 1) * TQ] = res.results[core]["out"]
    return out


# revision 37
# speedup vs baseline: 1.4547x; 1.4547x over previous
"""MultiHeadLatentAttention (MLA) Trainium2 Bass kernel, 8-core SPMD.

Sharding: cores 0-3 -> batch 0, cores 4-7 -> batch 1. Within a batch group,
core g owns query token rows [g*512, (g+1)*512).

Per core:
  phase 1 (own 512 tokens): transpose h -> hT; c_kv latent + roped k_r;
  q chain (c_q latent, q_c heads, roped q_r).
  AllGather (1.18MB) shares the *latent* c_kv + k_r within the batch group —
  issued after the q chain in program order so it overlaps it.
  attention: per head, K_c and V are recomputed from the gathered latent
  (16 matmuls each — cheaper than gathering 34MB of K/V), scores are built
  transposed [k, q] so softmax->PV needs no transposes; denominator via a
  ones-column matmul; W_o produces the core's own output rows directly.

All matmuls run in float32r (full PE rate, ~1.5e-4 rel err per matmul).
"""
import numpy as np
from contextlib import ExitStack

import ml_dtypes
import concourse.tile as tile
import concourse.mybir as mybir
from concourse import bacc
from concourse.bass_utils import run_bass_kernel_spmd
from concourse.masks import make_identity

P = 128
B, T = 2, 2048
D_MODEL, N_HEADS, D_HEAD = 2048, 16, 128
D_LATENT, D_LATENT_Q = 512, 1536
D_ROPE = 64
ROPE_BASE = 10000.0
N_CORES = 8
G = 4                      # cores per batch group
TQ = T // G                # 512 own query tokens per core
NKB = T // P               # 16 key blocks
SCALE = 1.0 / float(np.sqrt(D_HEAD + D_ROPE))

F32 = mybir.dt.float32
F32R = mybir.dt.float32r
BF16 = mybir.dt.bfloat16

CKV_SZ = D_LATENT * TQ         # 262144
KR_SZ = D_ROPE * TQ            # 32768
CC_SZ = CKV_SZ + KR_SZ


def _build(apply_mask: bool):
    nc = bacc.Bacc("TRN2", target_bir_lowering=False, debug=False,
                   num_devices=N_CORES)

    h_own = nc.dram_tensor("h_own", [TQ, D_MODEL], F32, kind="ExternalInput").ap()
    w_dq = nc.dram_tensor("w_dq", [D_MODEL, D_LATENT_Q], F32R, kind="ExternalInput").ap()
    w_uq = nc.dram_tensor("w_uq", [D_LATENT_Q, D_MODEL], F32R, kind="ExternalInput").ap()
    w_dkv = nc.dram_tensor("w_dkv", [D_MODEL, D_LATENT], F32R, kind="ExternalInput").ap()
    w_uk = nc.dram_tensor("w_uk", [D_LATENT, D_MODEL], F32R, kind="ExternalInput").ap()
    w_uv = nc.dram_tensor("w_uv", [D_LATENT, D_MODEL], F32R, kind="ExternalInput").ap()
    w_qr = nc.dram_tensor("w_qr", [D_LATENT_Q, N_HEADS * D_ROPE], F32R, kind="ExternalInput").ap()
    w_kr = nc.dram_tensor("w_kr", [D_MODEL, D_ROPE], F32R, kind="ExternalInput").ap()
    w_o = nc.dram_tensor("w_o", [D_MODEL, D_MODEL], F32R, kind="ExternalInput").ap()
    # cos/sin replicated 4x along partitions: row p holds table row p % 32
    cos_t = nc.dram_tensor("cos_t", [P, TQ], F32, kind="ExternalInput").ap()
    sin_t = nc.dram_tensor("sin_t", [P, TQ], F32, kind="ExternalInput").ap()
    if apply_mask:
        # mask[0,0][own_q_rows, :].T / SCALE, bf16, [k, q] layout
        mask_t = nc.dram_tensor("mask_t", [T, TQ], BF16, kind="ExternalInput").ap()
    out = nc.dram_tensor("out", [TQ, D_MODEL], F32, kind="ExternalOutput").ap()

    cc_in = nc.dram_tensor("cc_in", [CC_SZ], F32R)
    cc_ckv_out = nc.dram_tensor("cc_ckv_out", [G * CKV_SZ], F32R)
    cc_kr_out = nc.dram_tensor("cc_kr_out", [G * KR_SZ], F32R)
    qc_d = nc.dram_tensor("qc_d", [D_MODEL, TQ], F32R)   # q_c scratch, channel-major
    qro_d = nc.dram_tensor("qro_d", [2, 512, TQ], F32R)  # roped q_r scratch (x1; x2)

    with ExitStack() as ctx:
        tc = ctx.enter_context(tile.TileContext(nc))
        const = ctx.enter_context(tc.tile_pool(name="const", bufs=1))
        big = ctx.enter_context(tc.tile_pool(name="big", bufs=1))
        wpool = ctx.enter_context(tc.tile_pool(name="wpool", bufs=2))
        work = ctx.enter_context(tc.tile_pool(name="work", bufs=3))
        ps_proj = ctx.enter_context(tc.tile_pool(name="ps_proj", bufs=3, space="PSUM"))
        ps_sc = ctx.enter_context(tc.tile_pool(name="ps_sc", bufs=2, space="PSUM"))
        ps_den = ctx.enter_context(tc.tile_pool(name="ps_den", bufs=1, space="PSUM"))
        ps_pv = ctx.enter_context(tc.tile_pool(name="ps_pv", bufs=2, space="PSUM"))

        ident = const.tile([P, P], F32)
        make_identity(nc, ident[:])
        ones_f32 = const.tile([P, 1], F32)
        nc.vector.memset(ones_f32[:], 1.0)
        ones_col = const.tile([P, 1], F32R)
        nc.vector.tensor_copy(ones_col[:], ones_f32[:])
        ones_rf = const.tile([1, P], F32)
        nc.vector.memset(ones_rf[:], 1.0)
        ones_row = const.tile([1, P], F32R)
        nc.vector.tensor_copy(ones_row[:], ones_rf[:])
        cosb = const.tile([P, TQ], F32)
        nc.sync.dma_start(cosb[:], cos_t)
        sinb = const.tile([P, TQ], F32)
        nc.sync.dma_start(sinb[:], sin_t)

        # ---- load h in chunks and transpose to channel-major hT [2048, 512] ----
        hT = big.tile([P, D_MODEL // P, TQ], F32R, tag="hT")
        for i in range(TQ // P):
            h_sb = wpool.tile([P, D_MODEL], F32, tag="wA", name=f"h_sb_{i}")
            nc.sync.dma_start(h_sb[:], h_own[i * P:(i + 1) * P, :])
            for j in range(D_MODEL // P):
                pst = ps_proj.tile([P, P], F32, tag="proj", name=f"pst_{i}_{j}")
                nc.tensor.transpose(pst[:], h_sb[:, j * P:(j + 1) * P], ident[:])
                nc.vector.tensor_copy(hT[:, j, i * P:(i + 1) * P], pst[:])

        # generic projection: out_T[MD, TQ] = W[KD, MD].T @ actT   (lhsT = W)
        def project(w_ap, KD, MD, actT, out_cb, name):
            for m in range(MD // P):
                wt = wpool.tile([P, KD // P, P], F32R, tag="wA", name=f"w_{name}_{m}")
                nc.sync.dma_start(
                    wt[:], w_ap[:, m * P:(m + 1) * P].rearrange("(ko p) m -> p ko m", p=P))
                ps = ps_proj.tile([P, TQ], F32, tag="proj", name=f"ps_{name}_{m}")
                for k in range(KD // P):
                    nc.tensor.matmul(ps[:], wt[:, k], actT[:, k],
                                     start=(k == 0), stop=(k == KD // P - 1))
                out_cb(m, ps)

        # ---- own-token KV latent: k_r first (its gather is tiny; fire early)
        krt_raw = work.tile([D_ROPE, TQ], F32, tag="krr", bufs=1, name="krt_raw")
        wt_kr = wpool.tile([P, D_MODEL // P, D_ROPE], F32R, tag="wA", name="wkr")
        nc.sync.dma_start(wt_kr[:], w_kr.rearrange("(ko p) m -> p ko m", p=P))
        ps_kr = ps_proj.tile([D_ROPE, TQ], F32, tag="proj", name="ps_kr")
        for k in range(D_MODEL // P):
            nc.tensor.matmul(ps_kr[:], wt_kr[:, k], hT[:, k],
                             start=(k == 0), stop=(k == D_MODEL // P - 1))
        nc.vector.tensor_copy(krt_raw[:], ps_kr[:])
        k_rT = work.tile([D_ROPE, TQ], F32R, tag="qrs", bufs=1, name="k_rT")
        kx2 = work.tile([32, TQ], F32, tag="kx2", bufs=2)
        nc.sync.dma_start(kx2[:], krt_raw[32:64])
        kt1 = work.tile([32, TQ], F32, tag="rt_a", bufs=1)
        kt2 = work.tile([32, TQ], F32, tag="rt_b", bufs=1)
        ko2 = work.tile([32, TQ], F32R, tag="ro", bufs=2, name="ko2")
        nc.vector.tensor_mul(kt1[:], krt_raw[0:32], cosb[0:32])
        nc.vector.tensor_mul(kt2[:], kx2[:], sinb[0:32])
        nc.vector.tensor_sub(k_rT[0:32], kt1[:], kt2[:])
        nc.vector.tensor_mul(kt1[:], krt_raw[0:32], sinb[0:32])
        nc.vector.tensor_mul(kt2[:], kx2[:], cosb[0:32])
        nc.vector.tensor_add(ko2[:], kt1[:], kt2[:])
        nc.sync.dma_start(k_rT[32:64], ko2[:])
        nc.sync.dma_start(
            cc_in[CKV_SZ:CC_SZ].rearrange("(p t) -> p t", t=TQ), k_rT[:])
        nc.gpsimd.collective_compute(
            "AllGather", mybir.AluOpType.bypass,
            ins=[cc_in[CKV_SZ:CC_SZ]], outs=[cc_kr_out[:]],
            replica_groups=[[0, 1, 2, 3], [4, 5, 6, 7]],
        )

        c_kvT = big.tile([P, D_LATENT // P, TQ], F32R, tag="k8", bufs=(1 if apply_mask else 2))
        project(w_dkv, D_MODEL, D_LATENT, hT,
                lambda m, ps: nc.vector.tensor_copy(c_kvT[:, m], ps[:]), "dkv")
        nc.sync.dma_start(
            cc_in[0:CKV_SZ].rearrange("(o p t) -> p o t", p=P, t=TQ), c_kvT[:])
        nc.gpsimd.collective_compute(
            "AllGather", mybir.AluOpType.bypass,
            ins=[cc_in[0:CKV_SZ]], outs=[cc_ckv_out[:]],
            replica_groups=[[0, 1, 2, 3], [4, 5, 6, 7]],
        )

        # ---- Q chain ----
        c_qT = big.tile([P, D_LATENT_Q // P, TQ], F32R, tag="cqT")
        project(w_dq, D_MODEL, D_LATENT_Q, hT,
                lambda m, ps: nc.vector.tensor_copy(c_qT[:, m], ps[:]), "dq")

        def qc_out(m, ps):
            t = work.tile([P, TQ], F32R, tag="ro", name=f"qc_{m}", bufs=2)
            nc.vector.tensor_copy(t[:], ps[:])
            nc.sync.dma_start(qc_d[m * P:(m + 1) * P, :], t[:])
        project(w_uq, D_LATENT_Q, D_MODEL, c_qT, qc_out, "uq")

        # q_r: global x1/x2-split layout (host-permuted W_qr), rope on full tiles
        q_rT_raw = big.tile([P, (N_HEADS * D_ROPE) // P, TQ], F32, tag="qraw", bufs=2)
        project(w_qr, D_LATENT_Q, N_HEADS * D_ROPE, c_qT,
                lambda m, ps: nc.vector.tensor_copy(q_rT_raw[:, m], ps[:]), "qr")
        for kt in range(4):
            x1 = q_rT_raw[:, kt]
            x2 = q_rT_raw[:, 4 + kt]
            t1 = work.tile([P, TQ], F32, tag="rt_a", name=f"rt_a_{kt}", bufs=1)
            t2 = work.tile([P, TQ], F32, tag="rt_b", name=f"rt_b_{kt}", bufs=1)
            ro1 = work.tile([P, TQ], F32R, tag="ro", name=f"ro1_{kt}", bufs=2)
            ro2 = work.tile([P, TQ], F32R, tag="ro", name=f"ro2_{kt}", bufs=2)
            nc.vector.tensor_mul(t1[:], x1, cosb[:])
            nc.vector.tensor_mul(t2[:], x2, sinb[:])
            nc.vector.tensor_sub(ro1[:], t1[:], t2[:])
            nc.sync.dma_start(qro_d[0, kt * P:(kt + 1) * P, :], ro1[:])
            nc.vector.tensor_mul(t1[:], x1, sinb[:])
            nc.vector.tensor_mul(t2[:], x2, cosb[:])
            nc.vector.tensor_add(ro2[:], t1[:], t2[:])
            nc.sync.dma_start(qro_d[1, kt * P:(kt + 1) * P, :], ro2[:])

        # ---- load gathered latents ----
        ckvF = big.tile([P, D_LATENT // P, T], F32R, tag="hT", name="ckvF")
        krF = big.tile([D_ROPE, G, TQ], F32R, tag="krF")
        for r in range(G):
            nc.sync.dma_start(
                ckvF[:, :, r * TQ:(r + 1) * TQ],
                cc_ckv_out[r * CKV_SZ:(r + 1) * CKV_SZ].rearrange(
                    "(o p t) -> p o t", p=P, t=TQ))
            nc.sync.dma_start(
                krF[:, r, :],
                cc_kr_out[r * KR_SZ:(r + 1) * KR_SZ].rearrange(
                    "(p t) -> p t", t=TQ))

        if apply_mask:
            mask_sb = big.tile([P, T // P, TQ], BF16, tag="mask")
            nc.sync.dma_start(mask_sb[:], mask_t.rearrange("(o p) q -> p o q", p=P))

        attn_oT = big.tile([P, N_HEADS, TQ], F32R, tag="cqT", name="attn_oT")
        den_all = work.tile([N_HEADS, TQ], F32, tag="den_all", bufs=1)

        # ---- attention: 2-head groups share a V block ----
        for hg in range(N_HEADS // 2):
            wv = wpool.tile([P, D_LATENT // P, 256], F32R, tag="wA", name=f"wv_{hg}")
            nc.sync.dma_start(
                wv[:], w_uv[:, hg * 256:(hg + 1) * 256]
                .rearrange("(ko p) m -> p ko m", p=P))
            v2h = big.tile([P, NKB, 256], F32R, tag="qraw", name=f"v2h_{hg}", bufs=2)
            for m in range(NKB):
                ps = ps_proj.tile([P, 256], F32, tag="proj", name=f"ps_v_{hg}_{m}")
                for k in range(D_LATENT // P):
                    nc.tensor.matmul(ps[:], ckvF[:, k, m * P:(m + 1) * P], wv[:, k],
                                     start=(k == 0), stop=(k == D_LATENT // P - 1))
                nc.vector.tensor_copy(v2h[:, m], ps[:])

            for hh in range(2):
                h = 2 * hg + hh
                # K_c head block [128, 2048] from gathered latent
                wuk = wpool.tile([P, D_LATENT // P, P], F32R, tag="wA", name=f"wuk_{h}")
                nc.sync.dma_start(
                    wuk[:], w_uk[:, h * P:(h + 1) * P]
                    .rearrange("(ko p) m -> p ko m", p=P))
                kct = big.tile([P, T], F32R, tag="k8", name=f"kct_{h}", bufs=(1 if apply_mask else 2))
                for tcb in range(T // 512):
                    ps = ps_proj.tile([P, 512], F32, tag="proj", name=f"ps_kc_{h}_{tcb}")
                    for k in range(D_LATENT // P):
                        nc.tensor.matmul(ps[:], wuk[:, k],
                                         ckvF[:, k, tcb * 512:(tcb + 1) * 512],
                                         start=(k == 0), stop=(k == D_LATENT // P - 1))
                    nc.vector.tensor_copy(kct[:, tcb * 512:(tcb + 1) * 512], ps[:])

                qch = work.tile([P, TQ], F32R, tag="ro", name=f"qch_{h}", bufs=2)
                nc.sync.dma_start(qch[:], qc_d[h * P:(h + 1) * P, :])
                qr_stage = work.tile([D_ROPE, TQ], F32R, tag="qrs", name=f"qrs_{h}", bufs=1)
                nc.sync.dma_start(qr_stage[0:32], qro_d[0, h * 32:(h + 1) * 32, :])
                nc.sync.dma_start(qr_stage[32:64], qro_d[1, h * 32:(h + 1) * 32, :])

                pv_ps = ps_pv.tile([P, TQ], F32, tag="pv", name=f"pv_{h}")
                den_ps = ps_den.tile([1, TQ], F32, tag="den", name=f"den_{h}")
                for kb in range(NKB):
                    r, lc = kb // (TQ // P), (kb % (TQ // P)) * P
                    sc_ps = ps_sc.tile([P, TQ], F32, tag="sc", name=f"sc_{h}_{kb}")
                    nc.tensor.matmul(sc_ps[:], kct[:, kb * P:(kb + 1) * P], qch[:],
                                     start=True, stop=False)
                    nc.tensor.matmul(sc_ps[:], krF[:, r, lc:lc + P], qr_stage[:],
                                     start=False, stop=True)
                    if apply_mask:
                        nc.vector.tensor_add(sc_ps[:], sc_ps[:], mask_sb[:, kb])
                    pT = work.tile([P, TQ], F32R, tag="pT", name=f"pT_{h}_{kb}", bufs=2)
                    nc.scalar.activation(pT[:], sc_ps[:],
                                         mybir.ActivationFunctionType.Exp, scale=SCALE)
                    nc.tensor.matmul(den_ps[:], ones_col[:], pT[:],
                                     start=(kb == 0), stop=(kb == NKB - 1))
                    nc.tensor.matmul(pv_ps[:], v2h[:, kb, hh * P:(hh + 1) * P], pT[:],
                                     start=(kb == 0), stop=(kb == NKB - 1))
                den1 = work.tile([1, TQ], F32, tag="kx2", name=f"den1_{h}", bufs=2)
                nc.vector.tensor_copy(den1[:], den_ps[:])
                nc.sync.dma_start(den_all[h:h + 1, :], den1[:])
                nc.vector.tensor_copy(attn_oT[:, h], pv_ps[:])

        # ---- batched normalization: one reciprocal, then per-head scale ----
        recip_all = work.tile([N_HEADS, TQ], F32R, tag="recip", bufs=1)
        with nc.allow_low_precision(reason="f32r recip for PE broadcast"):
            nc.vector.reciprocal(recip_all[:], den_all[:])
        for h in range(N_HEADS):
            recip_h = work.tile([1, TQ], F32R, tag="recip_h", name=f"recip_h_{h}", bufs=1)
            nc.sync.dma_start(recip_h[:], recip_all[h:h + 1, :])
            den_bc = ps_sc.tile([P, TQ], F32, tag="sc", name=f"dbc_{h}")
            nc.tensor.matmul(den_bc[:], ones_row[:], recip_h[:],
                             start=True, stop=True)
            den_sb = work.tile([P, TQ], F32, tag="rt_a", name=f"densb_{h}", bufs=1)
            nc.vector.tensor_copy(den_sb[:], den_bc[:])
            nc.vector.tensor_mul(attn_oT[:, h], attn_oT[:, h].bitcast(F32), den_sb[:])

        # ---- W_o: out[tok, 2048] = attn_oT.T @ W_o (256-wide column slices) ----
        out_v = out.rearrange("(i p) c -> p i c", p=P)
        NW = 256
        for n in range(D_MODEL // NW):
            wo = wpool.tile([P, D_MODEL // P, NW], F32R, tag="qraw", name=f"wo_{n}", bufs=2)
            nc.sync.dma_start(
                wo[:], w_o[:, n * NW:(n + 1) * NW].rearrange("(ko p) m -> p ko m", p=P))
            for m in range(TQ // P):
                ps = ps_proj.tile([P, NW], F32, tag="proj", name=f"ps_o_{n}_{m}")
                for k in range(D_MODEL // P):
                    nc.tensor.matmul(ps[:], attn_oT[:, k, m * P:(m + 1) * P],
                                     wo[:, k], start=(k == 0), stop=(k == D_MODEL // P - 1))
                otile = work.tile([P, NW], F32, tag="rt_b", name=f"ot_{n}_{m}", bufs=1)
                nc.vector.tensor_copy(otile[:], ps[:])
                nc.sync.dma_start(out_v[:, m, n * NW:(n + 1) * NW], otile[:])

    nc.compile()
    return nc


def _rope_perm_q():
    # global: all heads' even rope dims (head-major), then all odd rope dims
    evens = [h * D_ROPE + 2 * j for h in range(N_HEADS) for j in range(D_ROPE // 2)]
    odds = [h * D_ROPE + 2 * j + 1 for h in range(N_HEADS) for j in range(D_ROPE // 2)]
    return np.array(evens + odds)


def _rope_perm_k():
    return np.array([2 * j for j in range(D_ROPE // 2)]
                    + [2 * j + 1 for j in range(D_ROPE // 2)])


def kernel(**inputs) -> np.ndarray:
    hidden = np.ascontiguousarray(np.asarray(inputs["hidden_states"], dtype=np.float32))
    mask = np.asarray(inputs["attention_mask"], dtype=np.float32)[0, 0]   # [T, T]
    apply_mask = bool(np.any(mask != 0.0))

    w_qr = np.ascontiguousarray(np.asarray(inputs["W_qr"], np.float32)[:, _rope_perm_q()])
    w_kr = np.ascontiguousarray(np.asarray(inputs["W_kr"], np.float32)[:, _rope_perm_k()])

    freqs = 1.0 / (ROPE_BASE ** (np.arange(0, D_ROPE, 2, dtype=np.float32) / D_ROPE))
    angles = np.outer(np.arange(T, dtype=np.float32), freqs)   # [T, 32]
    cos_full = np.tile(np.cos(angles).T.astype(np.float32), (4, 1))   # [128, T]
    sin_full = np.tile(np.sin(angles).T.astype(np.float32), (4, 1))

    shared = {
        "w_dq": np.ascontiguousarray(inputs["W_dq"], np.float32),
        "w_uq": np.ascontiguousarray(inputs["W_uq"], np.float32),
        "w_dkv": np.ascontiguousarray(inputs["W_dkv"], np.float32),
        "w_uk": np.ascontiguousarray(inputs["W_uk"], np.float32),
        "w_uv": np.ascontiguousarray(inputs["W_uv"], np.float32),
        "w_qr": w_qr,
        "w_kr": w_kr,
        "w_o": np.ascontiguousarray(inputs["W_o"], np.float32),
    }

    nc = _build(apply_mask)
    in_maps = []
    for core in range(N_CORES):
        b, g = core // G, core % G
        rows = slice(g * TQ, (g + 1) * TQ)
        m = dict(shared)
        m["h_own"] = np.ascontiguousarray(hidden[b, rows])
        m["cos_t"] = np.ascontiguousarray(cos_full[:, rows])
        m["sin_t"] = np.ascontiguousarray(sin_full[:, rows])
        if apply_mask:
            m["mask_t"] = np.ascontiguousarray(
                (mask[rows].T / SCALE).astype(ml_dtypes.bfloat16))
        in_maps.append(m)

    res = run_bass_kernel_spmd(nc, in_maps, core_ids=list(range(N_CORES)))

    out = np.empty((B, T, D_MODEL), dtype=np.float32)
    for core in range(N_CORES):
        b, g = core // G, core % G
        out[b, g * TQ:(g + 1) * TQ] = res.results[core]["out"]
    return out


# revision 40
# speedup vs baseline: 1.5485x; 1.0645x over previous
"""MultiHeadLatentAttention (MLA) Trainium2 Bass kernel, 8-core SPMD.

Sharding: cores 0-3 -> batch 0, cores 4-7 -> batch 1. Within a batch group,
core g owns query token rows [g*512, (g+1)*512).

Per core:
  phase 1 (own 512 tokens): transpose h -> hT; c_kv latent + roped k_r;
  q chain (c_q latent, q_c heads, roped q_r).
  AllGather (1.18MB) shares the *latent* c_kv + k_r within the batch group —
  issued after the q chain in program order so it overlaps it.
  attention: per head, K_c and V are recomputed from the gathered latent
  (16 matmuls each — cheaper than gathering 34MB of K/V), scores are built
  transposed [k, q] so softmax->PV needs no transposes; denominator via a
  ones-column matmul; W_o produces the core's own output rows directly.

All matmuls run in float32r (full PE rate, ~1.5e-4 rel err per matmul).
"""
import numpy as np
from contextlib import ExitStack

import ml_dtypes
import concourse.tile as tile
import concourse.mybir as mybir
from concourse import bacc
from concourse.bass_utils import run_bass_kernel_spmd
from concourse.masks import make_identity

P = 128
B, T = 2, 2048
D_MODEL, N_HEADS, D_HEAD = 2048, 16, 128
D_LATENT, D_LATENT_Q = 512, 1536
D_ROPE = 64
ROPE_BASE = 10000.0
N_CORES = 8
G = 4                      # cores per batch group
TQ = T // G                # 512 own query tokens per core
NKB = T // P               # 16 key blocks
SCALE = 1.0 / float(np.sqrt(D_HEAD + D_ROPE))

F32 = mybir.dt.float32
F32R = mybir.dt.float32r
BF16 = mybir.dt.bfloat16
FP8 = mybir.dt.float8e5

CKV_SZ = D_LATENT * TQ         # 262144
KR_SZ = D_ROPE * TQ            # 32768
CC_SZ = CKV_SZ + KR_SZ


def _build(apply_mask: bool, mask_fp8: bool = False):
    nc = bacc.Bacc("TRN2", target_bir_lowering=False, debug=False,
                   num_devices=N_CORES)

    h_own = nc.dram_tensor("h_own", [TQ, D_MODEL], F32, kind="ExternalInput").ap()
    w_dq = nc.dram_tensor("w_dq", [D_MODEL, D_LATENT_Q], F32R, kind="ExternalInput").ap()
    w_uq = nc.dram_tensor("w_uq", [D_LATENT_Q, D_MODEL], F32R, kind="ExternalInput").ap()
    w_dkv = nc.dram_tensor("w_dkv", [D_MODEL, D_LATENT], F32R, kind="ExternalInput").ap()
    w_uk = nc.dram_tensor("w_uk", [D_LATENT, D_MODEL], F32R, kind="ExternalInput").ap()
    w_uv = nc.dram_tensor("w_uv", [D_LATENT, D_MODEL], F32R, kind="ExternalInput").ap()
    w_qr = nc.dram_tensor("w_qr", [D_LATENT_Q, N_HEADS * D_ROPE], F32R, kind="ExternalInput").ap()
    w_kr = nc.dram_tensor("w_kr", [D_MODEL, D_ROPE], F32R, kind="ExternalInput").ap()
    w_o = nc.dram_tensor("w_o", [D_MODEL, D_MODEL], F32R, kind="ExternalInput").ap()
    # cos/sin replicated 4x along partitions: row p holds table row p % 32
    cos_t = nc.dram_tensor("cos_t", [P, TQ], F32, kind="ExternalInput").ap()
    sin_t = nc.dram_tensor("sin_t", [P, TQ], F32, kind="ExternalInput").ap()
    if apply_mask:
        # mask[0,0][own_q_rows, :].T / SCALE in [k, q] layout; fp8e5 when the
        # mask is binary (0 / -inf-like), bf16 otherwise
        mask_t = nc.dram_tensor("mask_t", [T, TQ], FP8 if mask_fp8 else BF16,
                                kind="ExternalInput").ap()
    out = nc.dram_tensor("out", [TQ, D_MODEL], F32, kind="ExternalOutput").ap()

    cc_in = nc.dram_tensor("cc_in", [CC_SZ], F32R)
    cc_ckv_out = nc.dram_tensor("cc_ckv_out", [G * CKV_SZ], F32R)
    cc_kr_out = nc.dram_tensor("cc_kr_out", [G * KR_SZ], F32R)
    qc_d = nc.dram_tensor("qc_d", [D_MODEL, TQ], F32R)   # q_c scratch, channel-major
    qro_d = nc.dram_tensor("qro_d", [2, 512, TQ], F32R)  # roped q_r scratch (x1; x2)

    with ExitStack() as ctx:
        tc = ctx.enter_context(tile.TileContext(nc))
        const = ctx.enter_context(tc.tile_pool(name="const", bufs=1))
        big = ctx.enter_context(tc.tile_pool(name="big", bufs=1))
        wpool = ctx.enter_context(tc.tile_pool(name="wpool", bufs=2))
        work = ctx.enter_context(tc.tile_pool(name="work", bufs=3))
        ps_proj = ctx.enter_context(tc.tile_pool(name="ps_proj", bufs=3, space="PSUM"))
        ps_sc = ctx.enter_context(tc.tile_pool(name="ps_sc", bufs=2, space="PSUM"))
        ps_den = ctx.enter_context(tc.tile_pool(name="ps_den", bufs=1, space="PSUM"))
        ps_pv = ctx.enter_context(tc.tile_pool(name="ps_pv", bufs=2, space="PSUM"))

        ident = const.tile([P, P], F32)
        make_identity(nc, ident[:])
        ones_f32 = const.tile([P, 1], F32)
        nc.vector.memset(ones_f32[:], 1.0)
        ones_col = const.tile([P, 1], F32R)
        nc.vector.tensor_copy(ones_col[:], ones_f32[:])
        ones_rf = const.tile([1, P], F32)
        nc.vector.memset(ones_rf[:], 1.0)
        ones_row = const.tile([1, P], F32R)
        nc.vector.tensor_copy(ones_row[:], ones_rf[:])
        cosb = const.tile([P, TQ], F32)
        nc.sync.dma_start(cosb[:], cos_t)
        sinb = const.tile([P, TQ], F32)
        nc.sync.dma_start(sinb[:], sin_t)

        # ---- load h in chunks and transpose to channel-major hT [2048, 512] ----
        hT = big.tile([P, D_MODEL // P, TQ], F32R, tag="hT")
        for i in range(TQ // P):
            h_sb = wpool.tile([P, D_MODEL], F32, tag="wA", name=f"h_sb_{i}")
            nc.sync.dma_start(h_sb[:], h_own[i * P:(i + 1) * P, :])
            for j in range(D_MODEL // P):
                pst = ps_proj.tile([P, P], F32, tag="proj", name=f"pst_{i}_{j}")
                nc.tensor.transpose(pst[:], h_sb[:, j * P:(j + 1) * P], ident[:])
                nc.vector.tensor_copy(hT[:, j, i * P:(i + 1) * P], pst[:])

        # generic projection: out_T[MD, TQ] = W[KD, MD].T @ actT   (lhsT = W)
        def project(w_ap, KD, MD, actT, out_cb, name):
            for m in range(MD // P):
                wt = wpool.tile([P, KD // P, P], F32R, tag="wA", name=f"w_{name}_{m}")
                nc.sync.dma_start(
                    wt[:], w_ap[:, m * P:(m + 1) * P].rearrange("(ko p) m -> p ko m", p=P))
                ps = ps_proj.tile([P, TQ], F32, tag="proj", name=f"ps_{name}_{m}")
                for k in range(KD // P):
                    nc.tensor.matmul(ps[:], wt[:, k], actT[:, k],
                                     start=(k == 0), stop=(k == KD // P - 1))
                out_cb(m, ps)

        # ---- own-token KV latent: k_r first (its gather is tiny; fire early)
        krt_raw = work.tile([D_ROPE, TQ], F32, tag="den_all", bufs=1, name="krt_raw")
        wt_kr = wpool.tile([P, D_MODEL // P, D_ROPE], F32R, tag="wA", name="wkr")
        nc.sync.dma_start(wt_kr[:], w_kr.rearrange("(ko p) m -> p ko m", p=P))
        ps_kr = ps_proj.tile([D_ROPE, TQ], F32, tag="proj", name="ps_kr")
        for k in range(D_MODEL // P):
            nc.tensor.matmul(ps_kr[:], wt_kr[:, k], hT[:, k],
                             start=(k == 0), stop=(k == D_MODEL // P - 1))
        nc.vector.tensor_copy(krt_raw[:], ps_kr[:])
        k_rT = work.tile([D_ROPE, TQ], F32R, tag="qrs", bufs=1, name="k_rT")
        kx2 = work.tile([32, TQ], F32, tag="kx2", bufs=2)
        nc.sync.dma_start(kx2[:], krt_raw[32:64])
        kt1 = work.tile([32, TQ], F32, tag="rt_a", bufs=1)
        kt2 = work.tile([32, TQ], F32, tag="rt_b", bufs=1)
        ko2 = work.tile([32, TQ], F32R, tag="ro", bufs=2, name="ko2")
        nc.vector.tensor_mul(kt1[:], krt_raw[0:32], cosb[0:32])
        nc.vector.tensor_mul(kt2[:], kx2[:], sinb[0:32])
        nc.vector.tensor_sub(k_rT[0:32], kt1[:], kt2[:])
        nc.vector.tensor_mul(kt1[:], krt_raw[0:32], sinb[0:32])
        nc.vector.tensor_mul(kt2[:], kx2[:], cosb[0:32])
        nc.vector.tensor_add(ko2[:], kt1[:], kt2[:])
        nc.sync.dma_start(k_rT[32:64], ko2[:])
        nc.sync.dma_start(
            cc_in[CKV_SZ:CC_SZ].rearrange("(p t) -> p t", t=TQ), k_rT[:])
        nc.gpsimd.collective_compute(
            "AllGather", mybir.AluOpType.bypass,
            ins=[cc_in[CKV_SZ:CC_SZ]], outs=[cc_kr_out[:]],
            replica_groups=[[0, 1, 2, 3], [4, 5, 6, 7]],
        )

        c_kvT = big.tile([P, D_LATENT // P, TQ], F32R, tag="k8", bufs=(1 if (apply_mask and not mask_fp8) else 2))
        project(w_dkv, D_MODEL, D_LATENT, hT,
                lambda m, ps: nc.vector.tensor_copy(c_kvT[:, m], ps[:]), "dkv")
        nc.sync.dma_start(
            cc_in[0:CKV_SZ].rearrange("(o p t) -> p o t", p=P, t=TQ), c_kvT[:])
        nc.gpsimd.collective_compute(
            "AllGather", mybir.AluOpType.bypass,
            ins=[cc_in[0:CKV_SZ]], outs=[cc_ckv_out[:]],
            replica_groups=[[0, 1, 2, 3], [4, 5, 6, 7]],
        )

        # ---- Q chain ----
        c_qT = big.tile([P, D_LATENT_Q // P, TQ], F32R, tag="cqT")
        project(w_dq, D_MODEL, D_LATENT_Q, hT,
                lambda m, ps: nc.vector.tensor_copy(c_qT[:, m], ps[:]), "dq")

        def qc_out(m, ps):
            t = work.tile([P, TQ], F32R, tag="ro", name=f"qc_{m}", bufs=2)
            nc.vector.tensor_copy(t[:], ps[:])
            nc.sync.dma_start(qc_d[m * P:(m + 1) * P, :], t[:])
        project(w_uq, D_LATENT_Q, D_MODEL, c_qT, qc_out, "uq")

        # q_r: global x1/x2-split layout (host-permuted W_qr), rope on full tiles
        q_rT_raw = big.tile([P, (N_HEADS * D_ROPE) // P, TQ], F32, tag="qraw", bufs=2)
        project(w_qr, D_LATENT_Q, N_HEADS * D_ROPE, c_qT,
                lambda m, ps: nc.vector.tensor_copy(q_rT_raw[:, m], ps[:]), "qr")
        for kt in range(4):
            x1 = q_rT_raw[:, kt]
            x2 = q_rT_raw[:, 4 + kt]
            t1 = work.tile([P, TQ], F32, tag="rt_a", name=f"rt_a_{kt}", bufs=1)
            t2 = work.tile([P, TQ], F32, tag="rt_b", name=f"rt_b_{kt}", bufs=1)
            ro1 = work.tile([P, TQ], F32R, tag="ro", name=f"ro1_{kt}", bufs=2)
            ro2 = work.tile([P, TQ], F32R, tag="ro", name=f"ro2_{kt}", bufs=2)
            nc.vector.tensor_mul(t1[:], x1, cosb[:])
            nc.vector.tensor_mul(t2[:], x2, sinb[:])
            nc.vector.tensor_sub(ro1[:], t1[:], t2[:])
            nc.sync.dma_start(qro_d[0, kt * P:(kt + 1) * P, :], ro1[:])
            nc.vector.tensor_mul(t1[:], x1, sinb[:])
            nc.vector.tensor_mul(t2[:], x2, cosb[:])
            nc.vector.tensor_add(ro2[:], t1[:], t2[:])
            nc.sync.dma_start(qro_d[1, kt * P:(kt + 1) * P, :], ro2[:])

        # ---- load gathered latents ----
        ckvF = big.tile([P, D_LATENT // P, T], F32R, tag="hT", name="ckvF")
        krF = big.tile([D_ROPE, G, TQ], F32R, tag="krF")
        for r in range(G):
            nc.sync.dma_start(
                ckvF[:, :, r * TQ:(r + 1) * TQ],
                cc_ckv_out[r * CKV_SZ:(r + 1) * CKV_SZ].rearrange(
                    "(o p t) -> p o t", p=P, t=TQ))
            nc.sync.dma_start(
                krF[:, r, :],
                cc_kr_out[r * KR_SZ:(r + 1) * KR_SZ].rearrange(
                    "(p t) -> p t", t=TQ))

        if apply_mask:
            mask_sb = big.tile([P, T // P, TQ], FP8 if mask_fp8 else BF16, tag="mask")
            nc.sync.dma_start(mask_sb[:], mask_t.rearrange("(o p) q -> p o q", p=P))

        attn_oT = big.tile([P, N_HEADS, TQ], F32R, tag="cqT", name="attn_oT")
        den_all = work.tile([N_HEADS, TQ], F32, tag="den_all", bufs=1, name="den_all")

        # ---- attention: 2-head groups share a V block ----
        for hg in range(N_HEADS // 2):
            wv = wpool.tile([P, D_LATENT // P, 256], F32R, tag="wA", name=f"wv_{hg}")
            nc.sync.dma_start(
                wv[:], w_uv[:, hg * 256:(hg + 1) * 256]
                .rearrange("(ko p) m -> p ko m", p=P))
            v2h = big.tile([P, NKB, 256], F32R, tag="qraw", name=f"v2h_{hg}", bufs=2)
            for m in range(NKB):
                ps = ps_proj.tile([P, 256], F32, tag="proj", name=f"ps_v_{hg}_{m}")
                for k in range(D_LATENT // P):
                    nc.tensor.matmul(ps[:], ckvF[:, k, m * P:(m + 1) * P], wv[:, k],
                                     start=(k == 0), stop=(k == D_LATENT // P - 1))
                nc.vector.tensor_copy(v2h[:, m], ps[:])

            for hh in range(2):
                h = 2 * hg + hh
                # K_c head block [128, 2048] from gathered latent
                wuk = wpool.tile([P, D_LATENT // P, P], F32R, tag="wA", name=f"wuk_{h}")
                nc.sync.dma_start(
                    wuk[:], w_uk[:, h * P:(h + 1) * P]
                    .rearrange("(ko p) m -> p ko m", p=P))
                kct = big.tile([P, T], F32R, tag="k8", name=f"kct_{h}", bufs=(1 if (apply_mask and not mask_fp8) else 2))
                for tcb in range(T // 512):
                    ps = ps_proj.tile([P, 512], F32, tag="proj", name=f"ps_kc_{h}_{tcb}")
                    for k in range(D_LATENT // P):
                        nc.tensor.matmul(ps[:], wuk[:, k],
                                         ckvF[:, k, tcb * 512:(tcb + 1) * 512],
                                         start=(k == 0), stop=(k == D_LATENT // P - 1))
                    nc.vector.tensor_copy(kct[:, tcb * 512:(tcb + 1) * 512], ps[:])

                qch = work.tile([P, TQ], F32R, tag="ro", name=f"qch_{h}", bufs=2)
                nc.sync.dma_start(qch[:], qc_d[h * P:(h + 1) * P, :])
                qr_stage = work.tile([D_ROPE, TQ], F32R, tag="qrs", name=f"qrs_{h}", bufs=1)
                nc.sync.dma_start(qr_stage[0:32], qro_d[0, h * 32:(h + 1) * 32, :])
                nc.sync.dma_start(qr_stage[32:64], qro_d[1, h * 32:(h + 1) * 32, :])

                pv_ps = ps_pv.tile([P, TQ], F32, tag="pv", name=f"pv_{h}")
                den_ps = ps_den.tile([1, TQ], F32, tag="den", name=f"den_{h}")
                for kb in range(NKB):
                    r, lc = kb // (TQ // P), (kb % (TQ // P)) * P
                    sc_ps = ps_sc.tile([P, TQ], F32, tag="sc", name=f"sc_{h}_{kb}")
                    nc.tensor.matmul(sc_ps[:], kct[:, kb * P:(kb + 1) * P], qch[:],
                                     start=True, stop=False)
                    nc.tensor.matmul(sc_ps[:], krF[:, r, lc:lc + P], qr_stage[:],
                                     start=False, stop=True)
                    if apply_mask:
                        nc.vector.tensor_add(sc_ps[:], sc_ps[:], mask_sb[:, kb])
                    pT = work.tile([P, TQ], F32R, tag="pT", name=f"pT_{h}_{kb}", bufs=2)
                    nc.scalar.activation(pT[:], sc_ps[:],
                                         mybir.ActivationFunctionType.Exp, scale=SCALE)
                    nc.tensor.matmul(den_ps[:], ones_col[:], pT[:],
                                     start=(kb == 0), stop=(kb == NKB - 1))
                    nc.tensor.matmul(pv_ps[:], v2h[:, kb, hh * P:(hh + 1) * P], pT[:],
                                     start=(kb == 0), stop=(kb == NKB - 1))
                den1 = work.tile([1, TQ], F32, tag="kx2", name=f"den1_{h}", bufs=2)
                nc.vector.tensor_copy(den1[:], den_ps[:])
                nc.sync.dma_start(den_all[h:h + 1, :], den1[:])
                nc.vector.tensor_copy(attn_oT[:, h], pv_ps[:])

        # ---- batched normalization: one reciprocal, then per-head scale ----
        recip_all = work.tile([N_HEADS, TQ], F32R, tag="recip", bufs=1)
        with nc.allow_low_precision(reason="f32r recip for PE broadcast"):
            nc.vector.reciprocal(recip_all[:], den_all[:])
        for h in range(N_HEADS):
            recip_h = work.tile([1, TQ], F32R, tag="recip_h", name=f"recip_h_{h}", bufs=1)
            nc.sync.dma_start(recip_h[:], recip_all[h:h + 1, :])
            den_bc = ps_sc.tile([P, TQ], F32, tag="sc", name=f"dbc_{h}")
            nc.tensor.matmul(den_bc[:], ones_row[:], recip_h[:],
                             start=True, stop=True)
            nc.vector.tensor_mul(attn_oT[:, h], attn_oT[:, h].bitcast(F32), den_bc[:])

        # ---- W_o: out[tok, 2048] = attn_oT.T @ W_o (256-wide column slices) ----
        out_v = out.rearrange("(i p) c -> p i c", p=P)
        NW = 256
        for n in range(D_MODEL // NW):
            wo = wpool.tile([P, D_MODEL // P, NW], F32R, tag="qraw", name=f"wo_{n}", bufs=2)
            nc.sync.dma_start(
                wo[:], w_o[:, n * NW:(n + 1) * NW].rearrange("(ko p) m -> p ko m", p=P))
            for m in range(TQ // P):
                ps = ps_proj.tile([P, NW], F32, tag="proj", name=f"ps_o_{n}_{m}")
                for k in range(D_MODEL // P):
                    nc.tensor.matmul(ps[:], attn_oT[:, k, m * P:(m + 1) * P],
                                     wo[:, k], start=(k == 0), stop=(k == D_MODEL // P - 1))
                otile = work.tile([P, NW], F32, tag="otile", name=f"ot_{n}_{m}", bufs=2)
                nc.vector.tensor_copy(otile[:], ps[:])
                nc.sync.dma_start(out_v[:, m, n * NW:(n + 1) * NW], otile[:])

    nc.compile()
    return nc


def _rope_perm_q():
    # global: all heads' even rope dims (head-major), then all odd rope dims
    evens = [h * D_ROPE + 2 * j for h in range(N_HEADS) for j in range(D_ROPE // 2)]
    odds = [h * D_ROPE + 2 * j + 1 for h in range(N_HEADS) for j in range(D_ROPE // 2)]
    return np.array(evens + odds)


def _rope_perm_k():
    return np.array([2 * j for j in range(D_ROPE // 2)]
                    + [2 * j + 1 for j in range(D_ROPE // 2)])


def kernel(**inputs) -> np.ndarray:
    hidden = np.ascontiguousarray(np.asarray(inputs["hidden_states"], dtype=np.float32))
    mask = np.asarray(inputs["attention_mask"], dtype=np.float32)[0, 0]   # [T, T]
    apply_mask = bool(np.any(mask != 0.0))
    # "binary" mask: entries are either 0 or very negative -> fp8 on device
    mask_fp8 = apply_mask and bool(
        np.all((mask == 0.0) | (mask < -1e6)))

    w_qr = np.ascontiguousarray(np.asarray(inputs["W_qr"], np.float32)[:, _rope_perm_q()])
    w_kr = np.ascontiguousarray(np.asarray(inputs["W_kr"], np.float32)[:, _rope_perm_k()])

    freqs = 1.0 / (ROPE_BASE ** (np.arange(0, D_ROPE, 2, dtype=np.float32) / D_ROPE))
    angles = np.outer(np.arange(T, dtype=np.float32), freqs)   # [T, 32]
    cos_full = np.tile(np.cos(angles).T.astype(np.float32), (4, 1))   # [128, T]
    sin_full = np.tile(np.sin(angles).T.astype(np.float32), (4, 1))

    shared = {
        "w_dq": np.ascontiguousarray(inputs["W_dq"], np.float32),
        "w_uq": np.ascontiguousarray(inputs["W_uq"], np.float32),
        "w_dkv": np.ascontiguousarray(inputs["W_dkv"], np.float32),
        "w_uk": np.ascontiguousarray(inputs["W_uk"], np.float32),
        "w_uv": np.ascontiguousarray(inputs["W_uv"], np.float32),
        "w_qr": w_qr,
        "w_kr": w_kr,
        "w_o": np.ascontiguousarray(inputs["W_o"], np.float32),
    }

    nc = _build(apply_mask, mask_fp8)
    in_maps = []
    for core in range(N_CORES):
        b, g = core // G, core % G
        rows = slice(g * TQ, (g + 1) * TQ)
        m = dict(shared)
        m["h_own"] = np.ascontiguousarray(hidden[b, rows])
        m["cos_t"] = np.ascontiguousarray(cos_full[:, rows])
        m["sin_t"] = np.ascontiguousarray(sin_full[:, rows])
        if apply_mask:
            mt = mask[rows].T / SCALE
            if mask_fp8:
                mt = np.where(mt < -1e6, -1280.0, mt)
                m["mask_t"] = np.ascontiguousarray(mt.astype(ml_dtypes.float8_e5m2))
            else:
                m["mask_t"] = np.ascontiguousarray(mt.astype(ml_dtypes.bfloat16))
        in_maps.append(m)

    res = run_bass_kernel_spmd(nc, in_maps, core_ids=list(range(N_CORES)))

    out = np.empty((B, T, D_MODEL), dtype=np.float32)
    for core in range(N_CORES):
        b, g = core // G, core % G
        out[b, g * TQ:(g + 1) * TQ] = res.results[core]["out"]
    return out
